# revision 4
# baseline (speedup 1.0000x reference)
"""BlockSparseAttention TRN2 kernel — 8-core SPMD (batch x head-half sharding).

Reference computation (B=4, S=2048, D=1024, H=16, Dh=64):
  q/k/v = x @ W{q,k,v}.T + b   -> [B,H,S,Dh]
  scores = q k^T / 8, masked to |i-j|<=32 plus global rows/cols (<4)
  out = softmax(scores) v  -> reassembled -> @ Wo.T + bo

Sharding: core = 2*b_half... core c handles batch b=c//2, head-group g=c%2
(heads 8g..8g+7, channels 512g..512g+511). Each core computes its heads'
attention output and a PARTIAL o-projection (contraction over its 512
channels); host sums the two partials per batch and adds bo.

On-chip layout is fully transposed ("T" = d-major): projections compute
q_T/k_T directly as [chan, s]; scores are computed transposed [t, s] so
softmax sums land in a matmul ones-row (v augmented with a ones column)
and no transposes are ever needed.
"""
import numpy as np
import ml_dtypes

import concourse.bass as bass
import concourse.bacc as bacc
import concourse.tile as tile
import concourse.mybir as mybir
from concourse.bass_utils import run_bass_kernel_spmd

F32 = mybir.dt.float32
BF16 = mybir.dt.bfloat16
AF = mybir.ActivationFunctionType
ALU = mybir.AluOpType

S = 2048
D = 1024
H = 16
DH = 64
NCORES = 8
NEG = -1.0e9
SCALE = 0.125

# s-tiles of 256 (8 of them); per tile the list of 128-aligned t-chunk offsets
# covering the sliding window [256r-32, 256r+288).
def chunk_plan():
    plans = []
    for r in range(8):
        if r == 0:
            plans.append([0, 128, 256])
        elif r == 7:
            plans.append([1664, 1792, 1920])
        else:
            plans.append([256 * r - 128, 256 * r, 256 * r + 128, 256 * r + 256])
    return plans


PLANS = chunk_plan()
NM = sum(len(p) for p in PLANS)  # 30 mask tiles


def build_nc():
    nc = bacc.Bacc()
    xT = nc.dram_tensor("xT", [128, 8, S], BF16, kind="ExternalInput")
    wq = nc.dram_tensor("wq", [128, 8, 512], BF16, kind="ExternalInput")
    wk = nc.dram_tensor("wk", [128, 8, 512], BF16, kind="ExternalInput")
    wv = nc.dram_tensor("wv", [128, 8, 512], BF16, kind="ExternalInput")
    wo = nc.dram_tensor("wo", [128, 4, 1024], BF16, kind="ExternalInput")
    bq_c = nc.dram_tensor("bq_c", [128, 4], F32, kind="ExternalInput")
    bk_c = nc.dram_tensor("bk_c", [128, 4], F32, kind="ExternalInput")
    bv_b = nc.dram_tensor("bv_b", [128, 512], F32, kind="ExternalInput")
    masks = nc.dram_tensor("masks", [128, NM, 256], BF16, kind="ExternalInput")
    out = nc.dram_tensor("out", [128, 8, S], F32, kind="ExternalOutput")

    with tile.TileContext(nc) as tc:
        with (
            tc.tile_pool(name="pers", bufs=1) as pers,
            tc.tile_pool(name="small", bufs=1) as small,
        ):
            q_sb = pers.tile([128, 4, S], BF16)
            k_sb = pers.tile([128, 4, S], BF16)
            v_sb = pers.tile([128, 16, 520], BF16)
            att_sb = pers.tile([128, 4, S], BF16)
            masks_sb = pers.tile([128, NM, 256], BF16)
            nc.sync.dma_start(out=masks_sb, in_=masks.ap())
            bq_sb = small.tile([128, 4], F32)
            bk_sb = small.tile([128, 4], F32)
            bv_sb = small.tile([128, 512], F32)
            nc.sync.dma_start(out=bq_sb, in_=bq_c.ap())
            nc.sync.dma_start(out=bk_sb, in_=bk_c.ap())
            nc.sync.dma_start(out=bv_sb, in_=bv_b.ap())

            # ---------------- Phase 1: projections ----------------
            with (
                tc.tile_pool(name="wpool", bufs=1) as wpool,
                tc.tile_pool(name="xpool", bufs=3) as xpool,
                tc.tile_pool(name="pproj", bufs=8, space="PSUM") as pproj,
            ):
                wq_sb = wpool.tile([128, 8, 512], BF16, tag="wq")
                wk_sb = wpool.tile([128, 8, 512], BF16, tag="wk")
                wv_sb = wpool.tile([128, 8, 512], BF16, tag="wv")
                nc.sync.dma_start(out=wq_sb, in_=wq.ap())
                nc.sync.dma_start(out=wk_sb, in_=wk.ap())
                nc.sync.dma_start(out=wv_sb, in_=wv.ap())

                for st in range(4):
                    ssl = slice(st * 512, (st + 1) * 512)
                    # pass A: q, k (chan-major psum [chan 128, s 512])
                    pq = [pproj.tile([128, 512], F32, tag="proj", name=f"pq{st}_{i}") for i in range(4)]
                    pk = [pproj.tile([128, 512], F32, tag="proj", name=f"pk{st}_{i}") for i in range(4)]
                    for dc in range(8):
                        xt = xpool.tile([128, 512], BF16, tag="xa")
                        nc.sync.dma_start(out=xt, in_=xT.ap()[:, dc, ssl])
                        for cb in range(4):
                            csl = slice(cb * 128, (cb + 1) * 128)
                            nc.tensor.matmul(
                                pq[cb], wq_sb[:, dc, csl], xt,
                                start=(dc == 0), stop=(dc == 7))
                            nc.tensor.matmul(
                                pk[cb], wk_sb[:, dc, csl], xt,
                                start=(dc == 0), stop=(dc == 7))
                    for cb in range(4):
                        nc.vector.tensor_scalar(
                            out=q_sb[:, cb, ssl], in0=pq[cb],
                            scalar1=bq_sb[:, cb:cb + 1], scalar2=None, op0=ALU.add)
                        nc.vector.tensor_scalar(
                            out=k_sb[:, cb, ssl], in0=pk[cb],
                            scalar1=bk_sb[:, cb:cb + 1], scalar2=None, op0=ALU.add)
                    # pass B: v (s-major psum [s 128, chan 512])
                    pv = [pproj.tile([128, 512], F32, tag="proj", name=f"pv{st}_{i}") for i in range(4)]
                    for dc in range(8):
                        xt = xpool.tile([128, 512], BF16, tag="xb")
                        nc.sync.dma_start(out=xt, in_=xT.ap()[:, dc, ssl])
                        for s4 in range(4):
                            nc.tensor.matmul(
                                pv[s4], xt[:, s4 * 128:(s4 + 1) * 128],
                                wv_sb[:, dc, :],
                                start=(dc == 0), stop=(dc == 7))
                    for s4 in range(4):
                        sc = st * 4 + s4
                        vview = v_sb[:, sc, :].rearrange("p (h w) -> p h w", h=8)
                        nc.vector.tensor_add(
                            out=vview[:, :, 0:64],
                            in0=pv[s4].rearrange("p (h w) -> p h w", h=8),
                            in1=bv_sb.rearrange("p (h w) -> p h w", h=8))
                        nc.vector.memset(vview[:, :, 64:65], 1.0)

            # ---------------- Phase 2: attention ----------------
            with (
                tc.tile_pool(name="psc", bufs=3, space="PSUM") as psc,
                tc.tile_pool(name="pmisc", bufs=2, space="PSUM") as pmisc,
                tc.tile_pool(name="pgrow", bufs=1, space="PSUM") as pgrow,
                tc.tile_pool(name="epool", bufs=4) as epool,
                tc.tile_pool(name="rpool", bufs=3) as rpool,
            ):
                midx = 0
                mask_base = {}
                for r in range(8):
                    mask_base[r] = midx
                    midx += len(PLANS[r])

                for r in range(8):
                    rsl = slice(r * 256, (r + 1) * 256)
                    for hp in range(4):
                        for hs in (0, 64):
                            h = hp * 2 + hs // 64
                            h65 = h * 65
                            hsl = slice(hs, hs + 64)
                            aug = pmisc.tile([65, 256], F32, tag="aug")
                            nav = 0
                            n_av_total = len(PLANS[r]) + (16 if r == 0 else 1)
                            for j, t0 in enumerate(PLANS[r]):
                                sct = psc.tile([128, 256], F32, tag="sc")
                                nc.tensor.matmul(
                                    sct, k_sb[hsl, hp, t0:t0 + 128],
                                    q_sb[hsl, hp, rsl], start=True, stop=True)
                                nc.vector.tensor_add(
                                    out=sct, in0=sct,
                                    in1=masks_sb[:, mask_base[r] + j, :])
                                ex = epool.tile([128, 256], BF16, tag="exp")
                                nc.scalar.activation(ex, sct, AF.Exp, scale=SCALE)
                                nc.tensor.matmul(
                                    aug, v_sb[:, t0 // 128, h65:h65 + 65], ex,
                                    start=(nav == 0), stop=(nav == n_av_total - 1))
                                nav += 1
                            if r == 0:
                                # global rows (s<4): full attention over all t
                                gsc = pgrow.tile([128, 64], F32, tag="grow")
                                for kk in range(16):
                                    nc.tensor.matmul(
                                        gsc[:, 4 * kk:4 * kk + 4],
                                        k_sb[hsl, hp, 128 * kk:128 * kk + 128],
                                        q_sb[hsl, hp, 0:4],
                                        start=(kk == 0), stop=(kk == 15))
                                exg = epool.tile([128, 64], BF16, tag="expg")
                                nc.scalar.activation(exg, gsc, AF.Exp, scale=SCALE)
                                for kk in range(16):
                                    nc.tensor.matmul(
                                        aug[:, 0:4], v_sb[:, kk, h65:h65 + 65],
                                        exg[:, 4 * kk:4 * kk + 4],
                                        start=False, stop=(nav == n_av_total - 1))
                                    nav += 1
                            else:
                                # global cols (t<4) for all rows of this s-tile
                                strip = pmisc.tile([4, 256], F32, tag="strip")
                                nc.tensor.matmul(
                                    strip, k_sb[hsl, hp, 0:4],
                                    q_sb[hsl, hp, rsl], start=True, stop=True)
                                exs = epool.tile([4, 256], BF16, tag="exps")
                                nc.scalar.activation(exs, strip, AF.Exp, scale=SCALE)
                                nc.tensor.matmul(
                                    aug[:, :], v_sb[0:4, 0, h65:h65 + 65], exs,
                                    start=False, stop=True)
                                nav += 1
                            # normalize: att = aug[0:64] * (1/aug[64])
                            rec = rpool.tile([1, 256], F32, tag="rec")
                            nc.vector.reciprocal(out=rec, in_=aug[64:65, :])
                            bc = rpool.tile([64, 256], F32, tag="bc")
                            nc.gpsimd.partition_broadcast(bc, rec)
                            nc.vector.tensor_mul(
                                out=att_sb[hsl, hp, rsl], in0=aug[0:64, :], in1=bc)

            # ---------------- Phase 3: output projection (partial) ----------------
            with (
                tc.tile_pool(name="wop", bufs=1) as wop,
                tc.tile_pool(name="opool", bufs=2) as opool,
                tc.tile_pool(name="pso", bufs=4, space="PSUM") as pso,
            ):
                wo_sb = wop.tile([128, 4, 1024], BF16)
                nc.sync.dma_start(out=wo_sb, in_=wo.ap())
                for et in range(8):
                    esl = slice(et * 128, (et + 1) * 128)
                    ot = opool.tile([128, S], F32)
                    for st in range(4):
                        ssl = slice(st * 512, (st + 1) * 512)
                        po = pso.tile([128, 512], F32, tag="po")
                        for cb in range(4):
                            nc.tensor.matmul(
                                po, wo_sb[:, cb, esl], att_sb[:, cb, ssl],
                                start=(cb == 0), stop=(cb == 3))
                        nc.vector.tensor_copy(out=ot[:, ssl], in_=po)
                    nc.sync.dma_start(out=out.ap()[:, et, :], in_=ot)

    nc.compile()
    return nc


def _host_masks():
    p = np.arange(128)[:, None]
    sl = np.arange(256)[None, :]
    tiles = np.empty((128, NM, 256), np.float32)
    i = 0
    for r in range(8):
        s = 256 * r + sl
        for t0 in PLANS[r]:
            t = t0 + p
            valid = (s >= 4) & ((np.abs(t - s) <= 32) | (t < 4))
            tiles[:, i, :] = np.where(valid, 0.0, NEG)
            i += 1
    return tiles.astype(ml_dtypes.bfloat16)


_NC = None
_LAST_IN_MAPS = None


def kernel(x, Wq, bq, Wk, bk, Wv, bv, Wo, bo):
    global _NC
    if _NC is None:
        _NC = build_nc()
    nc = _NC
    x = np.asarray(x, np.float32)
    B = x.shape[0]
    bf = ml_dtypes.bfloat16

    def chunked_T(a):  # [rows, cols] -> [128, rows//128, cols], a.T chunked
        at = np.ascontiguousarray(a.T)
        return at.reshape(at.shape[0] // 128, 128, at.shape[1]).transpose(1, 0, 2)

    masks_h = _host_masks()
    in_maps = []
    for core in range(NCORES):
        b, g = core // 2, core % 2
        gs = slice(512 * g, 512 * (g + 1))
        in_maps.append({
            "xT": np.ascontiguousarray(chunked_T(x[b])).astype(bf),
            "wq": np.ascontiguousarray(chunked_T(np.asarray(Wq)[gs, :])).astype(bf),
            "wk": np.ascontiguousarray(chunked_T(np.asarray(Wk)[gs, :])).astype(bf),
            "wv": np.ascontiguousarray(chunked_T(np.asarray(Wv)[gs, :])).astype(bf),
            "wo": np.ascontiguousarray(chunked_T(np.asarray(Wo)[:, gs])).astype(bf),
            "bq_c": np.asarray(bq)[gs].reshape(4, 128).T.copy().astype(np.float32),
            "bk_c": np.asarray(bk)[gs].reshape(4, 128).T.copy().astype(np.float32),
            "bv_b": np.broadcast_to(
                np.asarray(bv)[gs], (128, 512)).copy().astype(np.float32),
            "masks": masks_h,
        })

    global _LAST_IN_MAPS
    _LAST_IN_MAPS = in_maps
    res = run_bass_kernel_spmd(nc, in_maps, list(range(NCORES)))
    out = np.empty((B, S, D), np.float32)
    for b in range(B):
        acc = res.results[2 * b]["out"].astype(np.float32) + \
            res.results[2 * b + 1]["out"].astype(np.float32)
        # acc: [128, 8, S] with [p, et, s] = final_T[128*et+p, s]
        full_T = acc.transpose(1, 0, 2).reshape(D, S)
        out[b] = full_T.T + np.asarray(bo)[None, :]
    return out


# revision 7
# speedup vs baseline: 1.2428x; 1.2428x over previous
"""BlockSparseAttention TRN2 kernel — 8-core SPMD (batch x head-half sharding).

Reference computation (B=4, S=2048, D=1024, H=16, Dh=64):
  q/k/v = x @ W{q,k,v}.T + b   -> [B,H,S,Dh]
  scores = q k^T / 8, masked to |i-j|<=32 plus global rows/cols (<4)
  out = softmax(scores) v  -> reassembled -> @ Wo.T + bo

Sharding: core c handles batch b=c//2, head-group g=c%2 (heads 8g..8g+7,
channels 512g..512g+511). Each core computes its heads' attention output and
a PARTIAL o-projection (contraction over its 512 channels); host sums the two
partials per batch and adds bo.

On-chip layout is fully transposed (d-major): projections compute q_T/k_T
directly as [chan, s]; scores are computed transposed [t, s] so softmax sums
land in a matmul ones-row (v augmented with a ones column) and no transposes
are ever needed. Attention works on s-tiles of 256 with 128-aligned t-chunks;
within each chunk only the s-slice intersecting the band is computed.
"""
import numpy as np
import ml_dtypes

import concourse.bass as bass
import concourse.bacc as bacc
import concourse.tile as tile
import concourse.mybir as mybir
from concourse.bass_utils import run_bass_kernel_spmd

F32 = mybir.dt.float32
BF16 = mybir.dt.bfloat16
AF = mybir.ActivationFunctionType
ALU = mybir.AluOpType

S = 2048
D = 1024
NCORES = 8
NEG = -1.0e9
SCALE = 0.125


def chunk_plan():
    """Per s-tile r (256 rows): list of (t0, lo, w): 128-wide t-chunk at t0,
    contributing to local s columns [lo, lo+w)."""
    plans = []
    for r in range(8):
        if r == 0:
            t0s = [0, 128, 256]
        elif r == 7:
            t0s = [1664, 1792, 1920]
        else:
            t0s = [256 * r - 128, 256 * r, 256 * r + 128, 256 * r + 256]
        cur = []
        for j, t0 in enumerate(t0s):
            if r == 0 and j == 0:
                lo, hi = 0, 256  # global cols t<4 make every s valid
            else:
                lo = max(0, t0 - 32 - 256 * r)
                hi = min(256, t0 + 160 - 256 * r)
            cur.append((t0, lo, hi - lo))
        plans.append(cur)
    return plans


PLANS = chunk_plan()
MASK_OFF = []  # flat offsets into packed masks tensor, in (r, j) order
_off = 0
for _r in range(8):
    _row = []
    for (_t0, _lo, _w) in PLANS[_r]:
        _row.append(_off)
        _off += _w
    MASK_OFF.append(_row)
MASK_W = _off  # total packed width


def build_nc():
    nc = bacc.Bacc()
    xT = nc.dram_tensor("xT", [128, 8, S], BF16, kind="ExternalInput")
    wq = nc.dram_tensor("wq", [128, 8, 512], BF16, kind="ExternalInput")
    wk = nc.dram_tensor("wk", [128, 8, 512], BF16, kind="ExternalInput")
    wv = nc.dram_tensor("wv", [128, 8, 512], BF16, kind="ExternalInput")
    wo = nc.dram_tensor("wo", [128, 4, 1024], BF16, kind="ExternalInput")
    bq_c = nc.dram_tensor("bq_c", [128, 4], F32, kind="ExternalInput")
    bk_c = nc.dram_tensor("bk_c", [128, 4], F32, kind="ExternalInput")
    bv_b = nc.dram_tensor("bv_b", [128, 512], F32, kind="ExternalInput")
    masks = nc.dram_tensor("masks", [128, MASK_W], BF16, kind="ExternalInput")
    out = nc.dram_tensor("out", [128, 8, S], F32, kind="ExternalOutput")

    with tile.TileContext(nc) as tc:
        with (
            tc.tile_pool(name="pers", bufs=1) as pers,
            tc.tile_pool(name="small", bufs=1) as small,
        ):
            q_sb = pers.tile([128, 4, S], BF16)
            k_sb = pers.tile([128, 4, S], BF16)
            v_sb = pers.tile([128, 16, 520], BF16)
            att_sb = pers.tile([128, 4, S], BF16)
            masks_sb = pers.tile([128, MASK_W], BF16)
            nc.sync.dma_start(out=masks_sb, in_=masks.ap())
            bq_sb = small.tile([128, 4], F32)
            bk_sb = small.tile([128, 4], F32)
            bv_sb = small.tile([128, 512], F32)
            nc.sync.dma_start(out=bq_sb, in_=bq_c.ap())
            nc.sync.dma_start(out=bk_sb, in_=bk_c.ap())
            nc.sync.dma_start(out=bv_sb, in_=bv_b.ap())

            # ---------------- Phase 1: projections ----------------
            with (
                tc.tile_pool(name="wpool", bufs=1) as wpool,
                tc.tile_pool(name="xpool", bufs=3) as xpool,
                tc.tile_pool(name="pproj", bufs=8, space="PSUM") as pproj,
            ):
                wq_sb = wpool.tile([128, 8, 512], BF16, tag="wq")
                wk_sb = wpool.tile([128, 8, 512], BF16, tag="wk")
                wv_sb = wpool.tile([128, 8, 512], BF16, tag="wv")
                nc.sync.dma_start(out=wq_sb, in_=wq.ap())
                nc.sync.dma_start(out=wk_sb, in_=wk.ap())
                nc.sync.dma_start(out=wv_sb, in_=wv.ap())

                for st in range(4):
                    ssl = slice(st * 512, (st + 1) * 512)
                    pq = [pproj.tile([128, 512], F32, tag="proj", name=f"pq{st}_{i}")
                          for i in range(4)]
                    pk = [pproj.tile([128, 512], F32, tag="proj", name=f"pk{st}_{i}")
                          for i in range(4)]
                    for dc in range(8):
                        xt = xpool.tile([128, 512], BF16, tag="xa")
                        nc.sync.dma_start(out=xt, in_=xT.ap()[:, dc, ssl])
                        for cb in range(4):
                            csl = slice(cb * 128, (cb + 1) * 128)
                            nc.tensor.matmul(
                                pq[cb], wq_sb[:, dc, csl], xt,
                                start=(dc == 0), stop=(dc == 7))
                            nc.tensor.matmul(
                                pk[cb], wk_sb[:, dc, csl], xt,
                                start=(dc == 0), stop=(dc == 7))
                    for cb in range(4):
                        nc.vector.tensor_scalar(
                            out=q_sb[:, cb, ssl], in0=pq[cb],
                            scalar1=bq_sb[:, cb:cb + 1], scalar2=None, op0=ALU.add)
                        nc.vector.tensor_scalar(
                            out=k_sb[:, cb, ssl], in0=pk[cb],
                            scalar1=bk_sb[:, cb:cb + 1], scalar2=None, op0=ALU.add)
                    pv = [pproj.tile([128, 512], F32, tag="proj", name=f"pv{st}_{i}")
                          for i in range(4)]
                    for dc in range(8):
                        xt = xpool.tile([128, 512], BF16, tag="xb")
                        nc.sync.dma_start(out=xt, in_=xT.ap()[:, dc, ssl])
                        for s4 in range(4):
                            nc.tensor.matmul(
                                pv[s4], xt[:, s4 * 128:(s4 + 1) * 128],
                                wv_sb[:, dc, :],
                                start=(dc == 0), stop=(dc == 7))
                    for s4 in range(4):
                        sc = st * 4 + s4
                        vview = v_sb[:, sc, :].rearrange("p (h w) -> p h w", h=8)
                        nc.vector.tensor_add(
                            out=vview[:, :, 0:64],
                            in0=pv[s4].rearrange("p (h w) -> p h w", h=8),
                            in1=bv_sb.rearrange("p (h w) -> p h w", h=8))
                        nc.vector.memset(vview[:, :, 64:65], 1.0)

            # ---------------- Phase 2: attention ----------------
            with (
                tc.tile_pool(name="psc", bufs=3, space="PSUM") as psc,
                tc.tile_pool(name="paug", bufs=3, space="PSUM") as paug,
                tc.tile_pool(name="pmix", bufs=2, space="PSUM") as pmix,
                tc.tile_pool(name="epool", bufs=6) as epool,
                tc.tile_pool(name="rpool", bufs=4) as rpool,
            ):
                for r in range(8):
                    rsl = slice(r * 256, (r + 1) * 256)
                    for hp in range(4):
                        for hs in (0, 64):
                            h = hp * 2 + hs // 64
                            h65 = h * 65
                            hsl = slice(hs, hs + 64)
                            aug = paug.tile([65, 256], F32, tag="aug")
                            n_av = len(PLANS[r]) + (16 if r == 0 else 1)
                            nav = 0
                            if r > 0:
                                # global cols (t<4): full-width strip; its AV
                                # goes FIRST (start=True zeroes the aug bank).
                                strip = pmix.tile([4, 256], F32, tag="sg")
                                nc.tensor.matmul(
                                    strip, k_sb[hsl, hp, 0:4],
                                    q_sb[hsl, hp, rsl], start=True, stop=True)
                                exs = epool.tile([4, 256], BF16, tag="exps")
                                nc.scalar.activation(exs, strip, AF.Exp, scale=SCALE)
                                nc.tensor.matmul(
                                    aug, v_sb[0:4, 0, h65:h65 + 65], exs,
                                    start=True, stop=(nav == n_av - 1))
                                nav += 1
                            for j, (t0, lo, w) in enumerate(PLANS[r]):
                                ssl2 = slice(r * 256 + lo, r * 256 + lo + w)
                                sct = psc.tile([128, 256], F32, tag="sc")
                                nc.tensor.matmul(
                                    sct[:, 0:w], k_sb[hsl, hp, t0:t0 + 128],
                                    q_sb[hsl, hp, ssl2], start=True, stop=True)
                                mo = MASK_OFF[r][j]
                                nc.vector.tensor_add(
                                    out=sct[:, 0:w], in0=sct[:, 0:w],
                                    in1=masks_sb[:, mo:mo + w])
                                ex = epool.tile([128, 256], BF16, tag="exp")
                                nc.scalar.activation(
                                    ex[:, 0:w], sct[:, 0:w], AF.Exp, scale=SCALE)
                                nc.tensor.matmul(
                                    aug[:, lo:lo + w],
                                    v_sb[:, t0 // 128, h65:h65 + 65], ex[:, 0:w],
                                    start=(nav == 0), stop=(nav == n_av - 1))
                                nav += 1
                            if r == 0:
                                # global rows (s<4): full attention over all t
                                gsc = pmix.tile([128, 64], F32, tag="sg")
                                for kk in range(16):
                                    nc.tensor.matmul(
                                        gsc[:, 4 * kk:4 * kk + 4],
                                        k_sb[hsl, hp, 128 * kk:128 * kk + 128],
                                        q_sb[hsl, hp, 0:4],
                                        start=(kk == 0), stop=(kk == 15))
                                exg = epool.tile([128, 64], BF16, tag="expg")
                                nc.scalar.activation(exg, gsc, AF.Exp, scale=SCALE)
                                for kk in range(16):
                                    nc.tensor.matmul(
                                        aug[:, 0:4], v_sb[:, kk, h65:h65 + 65],
                                        exg[:, 4 * kk:4 * kk + 4],
                                        start=False, stop=(nav == n_av - 1))
                                    nav += 1
                            # unnormalized out -> att_sb (frees aug fast);
                            # then scale in place by 1/rowsum.
                            nc.scalar.copy(
                                out=att_sb[hsl, hp, rsl], in_=aug[0:64, :])
                            rec = rpool.tile([1, 256], F32, tag="rec")
                            nc.vector.reciprocal_approx_fast(
                                out=rec, in_=aug[64:65, :])
                            bc = rpool.tile([128, 256], F32, tag="bc")
                            nc.gpsimd.partition_broadcast(bc, rec)
                            nc.vector.tensor_mul(
                                out=att_sb[hsl, hp, rsl],
                                in0=att_sb[hsl, hp, rsl], in1=bc[hsl, :])

            # ------------- Phase 3: output projection (partial) -------------
            with (
                tc.tile_pool(name="wop", bufs=1) as wop,
                tc.tile_pool(name="opool", bufs=2) as opool,
                tc.tile_pool(name="pso", bufs=4, space="PSUM") as pso,
            ):
                wo_sb = wop.tile([128, 4, 1024], BF16)
                nc.sync.dma_start(out=wo_sb, in_=wo.ap())
                for et in range(8):
                    esl = slice(et * 128, (et + 1) * 128)
                    ot = opool.tile([128, S], F32)
                    for st in range(4):
                        ssl = slice(st * 512, (st + 1) * 512)
                        po = pso.tile([128, 512], F32, tag="po")
                        for cb in range(4):
                            nc.tensor.matmul(
                                po, wo_sb[:, cb, esl], att_sb[:, cb, ssl],
                                start=(cb == 0), stop=(cb == 3))
                        nc.vector.tensor_copy(out=ot[:, ssl], in_=po)
                    nc.sync.dma_start(out=out.ap()[:, et, :], in_=ot)

    nc.compile()
    return nc


def _host_masks():
    p = np.arange(128)[:, None]
    tiles = np.empty((128, MASK_W), np.float32)
    for r in range(8):
        for j, (t0, lo, w) in enumerate(PLANS[r]):
            sl = np.arange(lo, lo + w)[None, :]
            s = 256 * r + sl
            t = t0 + p
            valid = (s >= 4) & ((np.abs(t - s) <= 32) | (t < 4))
            mo = MASK_OFF[r][j]
            tiles[:, mo:mo + w] = np.where(valid, 0.0, NEG)
    return tiles.astype(ml_dtypes.bfloat16)


_NC = None
_LAST_IN_MAPS = None


def kernel(x, Wq, bq, Wk, bk, Wv, bv, Wo, bo):
    global _NC
    if _NC is None:
        _NC = build_nc()
    nc = _NC
    x = np.asarray(x, np.float32)
    B = x.shape[0]
    bf = ml_dtypes.bfloat16

    def chunked_T(a):  # [R, C] -> [128, C//128, R]; [p, c, r] = a[r, 128c+p]
        at = np.ascontiguousarray(a.T)
        return at.reshape(at.shape[0] // 128, 128, at.shape[1]).transpose(1, 0, 2)

    masks_h = _host_masks()
    in_maps = []
    for core in range(NCORES):
        b, g = core // 2, core % 2
        gs = slice(512 * g, 512 * (g + 1))
        in_maps.append({
            "xT": np.ascontiguousarray(chunked_T(x[b])).astype(bf),
            "wq": np.ascontiguousarray(chunked_T(np.asarray(Wq)[gs, :])).astype(bf),
            "wk": np.ascontiguousarray(chunked_T(np.asarray(Wk)[gs, :])).astype(bf),
            "wv": np.ascontiguousarray(chunked_T(np.asarray(Wv)[gs, :])).astype(bf),
            "wo": np.ascontiguousarray(chunked_T(np.asarray(Wo)[:, gs])).astype(bf),
            "bq_c": np.asarray(bq)[gs].reshape(4, 128).T.copy().astype(np.float32),
            "bk_c": np.asarray(bk)[gs].reshape(4, 128).T.copy().astype(np.float32),
            "bv_b": np.broadcast_to(
                np.asarray(bv)[gs], (128, 512)).copy().astype(np.float32),
            "masks": masks_h,
        })

    global _LAST_IN_MAPS
    _LAST_IN_MAPS = in_maps
    res = run_bass_kernel_spmd(nc, in_maps, list(range(NCORES)))
    out = np.empty((B, S, D), np.float32)
    for b in range(B):
        acc = res.results[2 * b]["out"].astype(np.float32) + \
            res.results[2 * b + 1]["out"].astype(np.float32)
        full_T = acc.transpose(1, 0, 2).reshape(D, S)
        out[b] = full_T.T + np.asarray(bo)[None, :]
    return out
